# revision 22
# baseline (speedup 1.0000x reference)
"""Multi-head self-attention (B=2, S=2048, E=1024, H=16) on 8 TRN2 NeuronCores.

Sharding: core c handles batch b=c//4 and head group g=c%4 (4 heads each).
 - QKV projections are head-sharded (each core computes Q/K/V only for its
   4 heads, over all 2048 tokens of its batch) -> no K/V exchange needed.
 - Attention (scores -> exp -> AV) is fully local per core.
 - Attention output is NORMALIZED on the sender (reciprocal of the fused
   rowsum, broadcast across partitions via a mask-valued rank-1 matmul that
   also zeroes wrong-batch copies), converted to bf16, and re-sharded
   from head-sharding to token-sharding with one AllToAll per HEAD-PAIR
   (2 collectives of [8,128,512] bf16 instead of 4 of [8,65,512] f32).
 - Output projection (Wo) then runs token-sharded, producing complete
   output rows; the host just transposes/concatenates.

Everything feature-major ("transposed") on device: x, Q, K are [dim, tok]
so the PE's partition-contraction works without any on-device transposes;
weights are pre-transposed on the host. V is tok-major for the AV matmul,
with a fused ones-column producing softmax row-sums for free.

Matmuls run in float32r (TF32, 1 cycle/row at moving free>=256, vs 4 for
fp32). Inputs are pre-rounded to TF32 on the host so DMA-loaded operands
are valid fp32r. Softmax skips the max-subtraction (logits are ~N(0,1),
bounded ~|6|, exp is safe in fp32) and folds the 1/sqrt(64) scale into the
ACT exp instruction; normalization is applied before the collective using
the fused row-sums. Wo loads and yt stores ride the (otherwise idle) Pool
DMA queue so phase-1 x loads start immediately.
"""

import numpy as np

import concourse.bass as bass
import concourse.mybir as mybir
from concourse import tile, bacc
from concourse.tile import add_dep_helper
from concourse.bass_utils import run_bass_kernel_spmd

B = 2
S = 2048
E = 1024
H = 16
DH = 64

NCORES = 8
GH = 4          # heads per core
GD = GH * DH    # 256 feature dims per core
TOK = S         # tokens per core (its whole batch element)
QB = 512        # q-block (moving free dim)
NQB = TOK // QB         # 4
NKT = TOK // 128        # 16 k-tiles
NET = E // 128          # 8 e-tiles
SCALE = 1.0 / np.sqrt(DH)

F32 = mybir.dt.float32
F32R = mybir.dt.float32r
BF16 = mybir.dt.bfloat16
FP = mybir.ActivationFunctionType
PAYLOAD_SCALE = 1.0    # unused now that the payload is bf16


def _tf32_round(a: np.ndarray) -> np.ndarray:
    b = np.ascontiguousarray(a, dtype=np.float32).view(np.uint32)
    r = (b + np.uint32(0x1000) + ((b >> np.uint32(13)) & np.uint32(1))) & np.uint32(0xFFFFE000)
    return r.view(np.float32)


MARKERS = {}


def build_nc(reps: int = 1):
    MARKERS.clear()
    nc = bacc.Bacc("TRN2", target_bir_lowering=False, debug=False, num_devices=NCORES)

    xt = nc.dram_tensor("xt", [E, TOK], F32R, kind="ExternalInput")       # x[b].T
    wqt = nc.dram_tensor("wqt", [E, GD], F32R, kind="ExternalInput")      # Wq.T cols for group
    wkt = nc.dram_tensor("wkt", [E, GD], F32R, kind="ExternalInput")
    wvt = nc.dram_tensor("wvt", [E, GD], F32R, kind="ExternalInput")
    wot = nc.dram_tensor("wot", [E, E], F32R, kind="ExternalInput")       # Wo.T full
    bq = nc.dram_tensor("bq", [GD], F32, kind="ExternalInput")
    bk = nc.dram_tensor("bk", [GD], F32, kind="ExternalInput")
    bv = nc.dram_tensor("bv", [GD], F32, kind="ExternalInput")
    bo = nc.dram_tensor("bo", [E], F32, kind="ExternalInput")
    # mask row [128]: cols 0:64 = 1.0 iff this core is batch-0, cols
    # 64:128 = 1.0 iff batch-1. lhsT of the rcp-broadcast matmul.
    mrow = nc.dram_tensor("mrow", [128], F32R, kind="ExternalInput")
    yt = nc.dram_tensor("yt", [E, QB], F32, kind="ExternalOutput")        # out rows, transposed

    with tile.TileContext(nc) as tc:
        with (
            tc.tile_pool(name="weights", bufs=1) as wp,
            tc.tile_pool(name="persist", bufs=1) as pp,
            tc.tile_pool(name="xt", bufs=2) as xp,
            tc.tile_pool(name="at", bufs=3) as ap_,
            tc.tile_pool(name="ot", bufs=4) as op_,
            tc.tile_pool(name="otf", bufs=1) as fp_,
            tc.tile_pool(name="yt", bufs=2) as yp,
            tc.tile_pool(name="dram", bufs=1, space="DRAM") as dp,
        ):
            # ---- persistent weights/biases in SBUF ----
            # qkv weights on the sync queue (needed first); wo on the Pool
            # queue so it never delays the phase-1 x loads.
            wq_t = [wp.tile([128, GD], F32R, name=f"wq{e}", tag=f"wq{e}") for e in range(NET)]
            wk_t = [wp.tile([128, GD], F32R, name=f"wk{e}", tag=f"wk{e}") for e in range(NET)]
            wv_t = [wp.tile([128, GD], F32R, name=f"wv{e}", tag=f"wv{e}") for e in range(NET)]
            wo_t = [wp.tile([128, E], F32R, name=f"wo{k}", tag=f"wo{k}") for k in range(NET)]
            for e in range(NET):
                nc.sync.dma_start(wk_t[e][:], wkt[e * 128:(e + 1) * 128, :])
                nc.sync.dma_start(wq_t[e][:], wqt[e * 128:(e + 1) * 128, :])
                nc.sync.dma_start(wv_t[e][:], wvt[e * 128:(e + 1) * 128, :])
            for e in range(NET):
                nc.gpsimd.dma_start(wo_t[e][:], wot[e * 128:(e + 1) * 128, :])

            bq_t = [pp.tile([128, 1], F32, name=f"bq{d}", tag=f"bq{d}") for d in range(2)]
            bk_t = [pp.tile([128, 1], F32, name=f"bk{d}", tag=f"bk{d}") for d in range(2)]
            bo_t = [pp.tile([128, 1], F32, name=f"bo{e}", tag=f"bo{e}") for e in range(NET)]
            for d in range(2):
                nc.sync.dma_start(
                    bq_t[d][:], bq[d * 128:(d + 1) * 128].rearrange("(p one) -> p one", one=1))
                nc.sync.dma_start(
                    bk_t[d][:], bk[d * 128:(d + 1) * 128].rearrange("(p one) -> p one", one=1))
            for e in range(NET):
                nc.gpsimd.dma_start(
                    bo_t[e][:], bo[e * 128:(e + 1) * 128].rearrange("(p one) -> p one", one=1))
            # bv broadcast across partitions: [GD] -> [128, GD]
            bv_t = pp.tile([128, GD], F32, name="bv", tag="bv")
            nc.gpsimd.dma_start(bv_t[:], bv.ap().partition_broadcast(128))
            ones_f32 = pp.tile([128, 64], F32, name="ones_f32", tag="ones_f32")
            nc.vector.memset(ones_f32[:], 1.0)
            # mask row for the rcp broadcast matmul: [1, 128]
            mrow_t = pp.tile([1, 128], F32R, name="mrow", tag="mrow")
            nc.sync.dma_start(mrow_t[:], mrow.rearrange("(one p) -> one p", one=1))

            # persistent activations
            qt_sb = [pp.tile([128, TOK], F32R, name=f"qt{d}", tag=f"qt{d}") for d in range(2)]
            kt_sb = [pp.tile([128, TOK], F32R, name=f"kt{d}", tag=f"kt{d}") for d in range(2)]
            # V tok-major, packed [v_h | 1] per head: 65 cols per head
            vp_sb = [pp.tile([128, GH * 65], F32R, name=f"vp{t}", tag=f"vp{t}") for t in range(NKT)]
            for t in range(NKT):
                for h in range(GH):
                    nc.vector.tensor_copy(
                        vp_sb[t][:, h * 65 + 64:h * 65 + 65], ones_f32[:, 0:1])

            # A2A bounce buffers, one per head: blocks [dest, 64, QB] bf16
            # of normalized O rows; dests 0:4 carry the batch-0 copy, 4:8
            # the batch-1 copy (the wrong-batch copy is zeroed via the mask
            # folded into the rcp broadcast; the receiver's add merges).
            a2a_in = [dp.tile([NCORES, 64, QB], BF16, name=f"a2ain{p}", tag=f"a2ain{p}")
                      for p in range(GH)]
            a2a_out = [dp.tile([NCORES, 64, QB], BF16, name=f"a2aout{p}", tag=f"a2aout{p}")
                       for p in range(GH)]

            def load_x(tb):
                # ACT's DMA queue is free while x loads (exp hasn't started
                # for the tokens being loaded).
                xts = []
                for e in range(NET):
                    xt_t = xp.tile([128, QB], F32R, name="xt", tag=f"xt{e}")
                    eng = nc.sync if (e % 2 == 0) else nc.scalar
                    eng.dma_start(xt_t[:], xt[e * 128:(e + 1) * 128, tb * QB:(tb + 1) * QB])
                    xts.append(xt_t)
                return xts

            def phase1():
                # K+V for all tokens complete first (x tiles recycled per
                # tb); then Q per q-block with x re-loaded (extra 8.4MB DMA
                # on otherwise idle queues) so attention starts earlier.
                with tc.tile_pool(name="ps_qkv", bufs=1, space="PSUM") as ps_qkv:
                    for tb in range(NQB):
                        xts = load_x(tb)
                        for d in range(2):
                            ps = ps_qkv.tile([128, QB], F32, name=f"k{d}", tag=f"k{d}")
                            for e in range(NET):
                                mm = nc.tensor.matmul(
                                    ps[:], wk_t[e][:, d * 128:(d + 1) * 128], xts[e][:],
                                    start=(e == 0), stop=(e == NET - 1))
                                if tb == 0 and d == 0 and e == 0:
                                    MARKERS.setdefault("p1_K_first", mm.ins.name)
                                if tb == NQB - 1 and d == 1 and e == NET - 1:
                                    MARKERS.setdefault("p1_K_last", mm.ins.name)
                            nc.vector.tensor_scalar_add(
                                kt_sb[d][:, tb * QB:(tb + 1) * QB], ps[:], bk_t[d][:])
                        # V: tok-major [tok, dh]
                        for vt in range(4):
                            ps = ps_qkv.tile([128, GD], F32, name=f"v{vt % 2}", tag=f"v{vt % 2}")
                            for e in range(NET):
                                nc.tensor.matmul(
                                    ps[:], xts[e][:, vt * 128:(vt + 1) * 128], wv_t[e][:],
                                    start=(e == 0), stop=(e == NET - 1))
                            t = tb * 4 + vt
                            dst2 = vp_sb[t][:].rearrange("p (h c) -> p h c", h=GH)[:, :, 0:64]
                            nc.vector.tensor_tensor(
                                dst2, ps[:].rearrange("p (h c) -> p h c", h=GH),
                                bv_t[:].rearrange("p (h c) -> p h c", h=GH),
                                op=mybir.AluOpType.add)
                    for tb in range(NQB):
                        xts = load_x(tb)
                        for d in range(2):
                            ps = ps_qkv.tile([128, QB], F32, name=f"q{d}", tag=f"q{d}")
                            for e in range(NET):
                                mm = nc.tensor.matmul(
                                    ps[:], wq_t[e][:, d * 128:(d + 1) * 128], xts[e][:],
                                    start=(e == 0), stop=(e == NET - 1))
                                if tb == 0 and d == 1 and e == NET - 1:
                                    MARKERS.setdefault("p1_Q0_last", mm.ins.name)
                            nc.vector.tensor_scalar_add(
                                qt_sb[d][:, tb * QB:(tb + 1) * QB], ps[:], bq_t[d][:])

            prev_cc = {}
            prev_rd = {}

            def phase2():
                # per-head attention; each head's normalized bf16 output is
                # re-sharded head->token with its own small AllToAll so the
                # collectives start (and finish) as early as possible.
                ccs = {}
                with (
                    tc.tile_pool(name="ps_s", bufs=2, space="PSUM") as ps_s,
                    tc.tile_pool(name="ps_av", bufs=2, space="PSUM") as ps_av,
                    tc.tile_pool(name="ps_rb", bufs=1, space="PSUM") as ps_rb,
                ):
                    # k-tile groups sized to the scores psum tile (2 banks)
                    GRPS = [(k, 2) for k in range(0, NKT, 2)]
                    for h in range(GH):
                        writers = []
                        d, p0 = h // 2, (h % 2) * 64
                        for qb in range(NQB):
                            av_ps = ps_av.tile([65, QB], F32, name="av", tag="av")
                            for g0, gn in GRPS:
                                s_ps = ps_s.tile([128, 2 * QB], F32, name="s", tag="s")
                                for ki in range(gn):
                                    kt = g0 + ki
                                    nc.tensor.matmul(
                                        s_ps[:, ki * QB:(ki + 1) * QB],
                                        kt_sb[d][p0:p0 + 64, kt * 128:(kt + 1) * 128],
                                        qt_sb[d][p0:p0 + 64, qb * QB:(qb + 1) * QB],
                                        start=True, stop=True)
                                at_t = ap_.tile([128, 2 * QB], F32R, name="at", tag="at")
                                act = nc.scalar.activation(
                                    at_t[:, 0:gn * QB], s_ps[:, 0:gn * QB],
                                    FP.Exp, scale=float(SCALE))
                                MARKERS.setdefault(f"p2_exp_first_h{h}_q{qb}", act.ins.name)
                                for ki in range(gn):
                                    kt = g0 + ki
                                    nc.tensor.matmul(
                                        av_ps[:],
                                        vp_sb[kt][:, h * 65:h * 65 + 65],
                                        at_t[:, ki * QB:(ki + 1) * QB],
                                        start=(kt == 0), stop=(kt == NKT - 1))
                            # copy the accumulator to SBUF (frees the PSUM
                            # bank early, and TensorTensor may read at most
                            # one PSUM operand on hardware).
                            av_sb = op_.tile([65, QB], F32R, name="av_sb", tag="av_sb")
                            with nc.allow_low_precision(reason="tf32 av copy"):
                                nc.vector.tensor_copy(av_sb[:], av_ps[:])
                            # normalize + batch-mask on the sender:
                            # rcp = 1/rowsum [1, QB]; rb = mrow.T @ rcp
                            # [128, QB]: rows 0:64 = mlo*rcp, 64:128 =
                            # mhi*rcp (one of them all-zero).
                            rcp_t = op_.tile([1, QB], F32R, name="rcp", tag="rcp")
                            with nc.allow_low_precision(reason="tf32 rcp"):
                                nc.vector.reciprocal(rcp_t[:], av_sb[64:65, :])
                            rb_ps = ps_rb.tile([128, QB], F32, name="rb", tag="rb")
                            nc.tensor.matmul(
                                rb_ps[:], mrow_t[:], rcp_t[:], start=True, stop=True)
                            ot_lo = op_.tile([64, QB], BF16, name="ot_lo", tag="ot_lo")
                            ot_hi = op_.tile([64, QB], BF16, name="ot_hi", tag="ot_hi")
                            with nc.allow_low_precision(reason="bf16 a2a payload"):
                                nc.vector.tensor_tensor(
                                    ot_lo[:], av_sb[0:64, :], rb_ps[0:64, :],
                                    op=mybir.AluOpType.mult)
                                nc.vector.tensor_tensor(
                                    ot_hi[:], av_sb[0:64, :], rb_ps[64:128, :],
                                    op=mybir.AluOpType.mult)
                            w1 = nc.sync.dma_start(a2a_in[h][qb, :, :], ot_lo[:])
                            w2 = nc.sync.dma_start(a2a_in[h][qb + 4, :, :], ot_hi[:])
                            if h in prev_cc:
                                add_dep_helper(w1.ins, prev_cc[h].ins,
                                               reason="a2a_in reuse waits prior collective")
                                add_dep_helper(w2.ins, prev_cc[h].ins,
                                               reason="a2a_in reuse waits prior collective")
                            writers.append(w1)
                            writers.append(w2)
                        cc = nc.gpsimd.collective_compute(
                            "AllToAll", mybir.AluOpType.bypass,
                            replica_groups=[list(range(NCORES))],
                            ins=[a2a_in[h].opt()], outs=[a2a_out[h].opt()])
                        MARKERS.setdefault(f"p2_cc{h}", cc.ins.name)
                        for w in writers:
                            add_dep_helper(cc.ins, w.ins, reason="collective waits on a2a input writes")
                        for rdp in prev_rd.get(h, ()):
                            add_dep_helper(cc.ins, rdp.ins,
                                           reason="a2a_out overwrite waits prior reads")
                        ccs[h] = cc
                        prev_cc[h] = cc
                return ccs

            def phase3(ccs):
                with tc.tile_pool(name="ps_y", bufs=2, space="PSUM") as ps_y:
                    # otf_t[k] = O.T dims 128k..128k+127 for own tokens =
                    # local heads (2*(k%2), 2*(k%2)+1) from source group
                    # k//2; merge the two batch copies (wrong one is zero).
                    otf_t = [fp_.tile([128, QB], F32R, name=f"otf{k}", tag=f"otf{k}") for k in range(NET)]
                    korder = [0, 2, 4, 6, 1, 3, 5, 7]
                    prev_rd.clear()
                    for k in korder:
                        g_src = k // 2
                        for half in range(2):
                            h = 2 * (k % 2) + half
                            ou_lo = fp_.tile([64, QB], BF16, name="ou_lo", tag="ou_lo", bufs=2)
                            ou_hi = fp_.tile([64, QB], BF16, name="ou_hi", tag="ou_hi", bufs=2)
                            rd1 = nc.gpsimd.dma_start(ou_lo[:], a2a_out[h][g_src, :, :])
                            rd2 = nc.gpsimd.dma_start(ou_hi[:], a2a_out[h][4 + g_src, :, :])
                            add_dep_helper(rd1.ins, ccs[h].ins, reason="otf read waits on collective")
                            add_dep_helper(rd2.ins, ccs[h].ins, reason="otf read waits on collective")
                            prev_rd.setdefault(h, []).extend([rd1, rd2])
                            with nc.allow_low_precision(reason="bf16 -> tf32 merge"):
                                nc.vector.tensor_tensor(
                                    otf_t[k][half * 64:(half + 1) * 64, :],
                                    ou_lo[:], ou_hi[:], op=mybir.AluOpType.add)
                    for e in range(NET):
                        ps = ps_y.tile([128, QB], F32, name="y", tag="y")
                        for i, k in enumerate(korder):
                            mm = nc.tensor.matmul(
                                ps[:], wo_t[k][:, e * 128:(e + 1) * 128], otf_t[k][:],
                                start=(i == 0), stop=(i == NET - 1))
                            if e == 0 and i == 0:
                                MARKERS.setdefault("p3_fc_first", mm.ins.name)
                        y_t = yp.tile([128, QB], F32, name="yt", tag="yt")
                        nc.vector.tensor_scalar_add(y_t[:], ps[:], bo_t[e][:])
                        w = nc.gpsimd.dma_start(yt[e * 128:(e + 1) * 128, :], y_t[:])
                        if e == NET - 1:
                            MARKERS.setdefault("p3_yt_last", w.ins.name)

            # Software pipeline across reps: rep r+1's QKV projections are
            # emitted BEFORE rep r's output projection so the PE fills the
            # collective tail with next-rep work.
            phase1()
            for r in range(reps):
                ccs = phase2()
                if r + 1 < reps:
                    phase1()
                phase3(ccs)

    nc.compile()
    return nc


_CACHE = {}


def _get_nc(reps: int = 1):
    if reps not in _CACHE:
        _CACHE[reps] = build_nc(reps)
    return _CACHE[reps]


def make_in_maps(x, Wq, bq, Wk, bk, Wv, bv, Wo, bo):
    x = np.asarray(x, np.float32)
    xts = [_tf32_round(np.ascontiguousarray(x[b].T)) for b in range(B)]
    wqt = _tf32_round(np.ascontiguousarray(np.asarray(Wq, np.float32).T))
    wkt = _tf32_round(np.ascontiguousarray(np.asarray(Wk, np.float32).T))
    wvt = _tf32_round(np.ascontiguousarray(np.asarray(Wv, np.float32).T))
    # Wo pre-divided by PAYLOAD_SCALE: the a2a payload ships O*PAYLOAD_SCALE
    wot = _tf32_round(np.ascontiguousarray(np.asarray(Wo, np.float32).T / PAYLOAD_SCALE))
    bq = np.asarray(bq, np.float32); bk = np.asarray(bk, np.float32)
    bv = np.asarray(bv, np.float32); bo = np.asarray(bo, np.float32)
    in_maps = []
    for c in range(NCORES):
        b, g = c // 4, c % 4
        sl = slice(g * GD, (g + 1) * GD)
        mrow = np.zeros(128, np.float32)
        mrow[0:64] = 1.0 if b == 0 else 0.0
        mrow[64:128] = 1.0 if b == 1 else 0.0
        in_maps.append({
            "mrow": mrow,
            "xt": xts[b],
            "wqt": np.ascontiguousarray(wqt[:, sl]),
            "wkt": np.ascontiguousarray(wkt[:, sl]),
            "wvt": np.ascontiguousarray(wvt[:, sl]),
            "wot": wot,
            "bq": np.ascontiguousarray(bq[sl]),
            "bk": np.ascontiguousarray(bk[sl]),
            "bv": np.ascontiguousarray(bv[sl]),
            "bo": bo,
        })
    return in_maps


def kernel(x, Wq, bq, Wk, bk, Wv, bv, Wo, bo):
    nc = _get_nc(1)
    in_maps = make_in_maps(x, Wq, bq, Wk, bk, Wv, bv, Wo, bo)
    res = run_bass_kernel_spmd(nc, in_maps, list(range(NCORES)))
    out = np.empty((B, S, E), np.float32)
    for c in range(NCORES):
        b, g = c // 4, c % 4
        out[b, g * QB:(g + 1) * QB, :] = res.results[c]["yt"].T
    return out


# revision 23
# speedup vs baseline: 2.3977x; 2.3977x over previous
"""Multi-head self-attention (B=2, S=2048, E=1024, H=16) on 8 TRN2 NeuronCores.

Sharding: core c handles batch b=c//4 and head group g=c%4 (4 heads each).
 - QKV projections are head-sharded (each core computes Q/K/V only for its
   4 heads, over all 2048 tokens of its batch) -> no K/V exchange needed.
 - Attention (scores -> exp -> AV) is fully local per core.
 - Attention output is NORMALIZED on the sender (reciprocal of the fused
   rowsum, broadcast across partitions via a mask-valued rank-1 matmul that
   also zeroes wrong-batch copies), converted to bf16, and re-sharded
   from head-sharding to token-sharding with one AllToAll per HEAD-PAIR
   (2 collectives of [8,128,512] bf16 instead of 4 of [8,65,512] f32).
 - Output projection (Wo) then runs token-sharded, producing complete
   output rows; the host just transposes/concatenates.

Everything feature-major ("transposed") on device: x, Q, K are [dim, tok]
so the PE's partition-contraction works without any on-device transposes;
weights are pre-transposed on the host. V is tok-major for the AV matmul,
with a fused ones-column producing softmax row-sums for free.

Matmuls run in float32r (TF32, 1 cycle/row at moving free>=256, vs 4 for
fp32). Inputs are pre-rounded to TF32 on the host so DMA-loaded operands
are valid fp32r. Softmax skips the max-subtraction (logits are ~N(0,1),
bounded ~|6|, exp is safe in fp32) and folds the 1/sqrt(64) scale into the
ACT exp instruction; normalization is applied before the collective using
the fused row-sums. Wo loads and yt stores ride the (otherwise idle) Pool
DMA queue so phase-1 x loads start immediately.
"""

import numpy as np

import concourse.bass as bass
import concourse.mybir as mybir
from concourse import tile, bacc
from concourse.tile import add_dep_helper
from concourse.bass_utils import run_bass_kernel_spmd

B = 2
S = 2048
E = 1024
H = 16
DH = 64

NCORES = 8
GH = 4          # heads per core
GD = GH * DH    # 256 feature dims per core
TOK = S         # tokens per core (its whole batch element)
QB = 512        # q-block (moving free dim)
NQB = TOK // QB         # 4
NKT = TOK // 128        # 16 k-tiles
NET = E // 128          # 8 e-tiles
SCALE = 1.0 / np.sqrt(DH)

F32 = mybir.dt.float32
F32R = mybir.dt.float32r
BF16 = mybir.dt.bfloat16
FP = mybir.ActivationFunctionType
PAYLOAD_SCALE = 1.0    # unused now that the payload is bf16


def _tf32_round(a: np.ndarray) -> np.ndarray:
    b = np.ascontiguousarray(a, dtype=np.float32).view(np.uint32)
    r = (b + np.uint32(0x1000) + ((b >> np.uint32(13)) & np.uint32(1))) & np.uint32(0xFFFFE000)
    return r.view(np.float32)


MARKERS = {}


def build_nc(reps: int = 1):
    MARKERS.clear()
    nc = bacc.Bacc("TRN2", target_bir_lowering=False, debug=False, num_devices=NCORES)

    xt = nc.dram_tensor("xt", [E, TOK], F32R, kind="ExternalInput")       # x[b].T
    wqt = nc.dram_tensor("wqt", [E, GD], F32R, kind="ExternalInput")      # Wq.T cols for group
    wkt = nc.dram_tensor("wkt", [E, GD], F32R, kind="ExternalInput")
    wvt = nc.dram_tensor("wvt", [E, GD], F32R, kind="ExternalInput")
    wot = nc.dram_tensor("wot", [E, E], F32R, kind="ExternalInput")       # Wo.T full
    bq = nc.dram_tensor("bq", [GD], F32, kind="ExternalInput")
    bk = nc.dram_tensor("bk", [GD], F32, kind="ExternalInput")
    bv = nc.dram_tensor("bv", [GD], F32, kind="ExternalInput")
    bo = nc.dram_tensor("bo", [E], F32, kind="ExternalInput")
    # mask row [128]: cols 0:64 = 1.0 iff this core is batch-0, cols
    # 64:128 = 1.0 iff batch-1. lhsT of the rcp-broadcast matmul.
    mrow = nc.dram_tensor("mrow", [128], F32R, kind="ExternalInput")
    yt = nc.dram_tensor("yt", [E, QB], F32, kind="ExternalOutput")        # out rows, transposed

    with tile.TileContext(nc) as tc:
        with (
            tc.tile_pool(name="weights", bufs=1) as wp,
            tc.tile_pool(name="persist", bufs=1) as pp,
            tc.tile_pool(name="xt", bufs=2) as xp,
            tc.tile_pool(name="at", bufs=3) as ap_,
            tc.tile_pool(name="ot", bufs=4) as op_,
            tc.tile_pool(name="otf", bufs=1) as fp_,
            tc.tile_pool(name="yt", bufs=2) as yp,
            tc.tile_pool(name="dram", bufs=1, space="DRAM") as dp,
        ):
            # ---- persistent weights/biases in SBUF ----
            # qkv weights on the sync queue (needed first); wo on the Pool
            # queue so it never delays the phase-1 x loads.
            wq_t = [wp.tile([128, GD], F32R, name=f"wq{e}", tag=f"wq{e}") for e in range(NET)]
            wk_t = [wp.tile([128, GD], F32R, name=f"wk{e}", tag=f"wk{e}") for e in range(NET)]
            wv_t = [wp.tile([128, GD], F32R, name=f"wv{e}", tag=f"wv{e}") for e in range(NET)]
            wo_t = [wp.tile([128, E], F32R, name=f"wo{k}", tag=f"wo{k}") for k in range(NET)]
            for e in range(NET):
                nc.sync.dma_start(wk_t[e][:], wkt[e * 128:(e + 1) * 128, :])
                nc.sync.dma_start(wq_t[e][:], wqt[e * 128:(e + 1) * 128, :])
                nc.sync.dma_start(wv_t[e][:], wvt[e * 128:(e + 1) * 128, :])
            for e in range(NET):
                nc.gpsimd.dma_start(wo_t[e][:], wot[e * 128:(e + 1) * 128, :])

            bq_t = [pp.tile([128, 1], F32, name=f"bq{d}", tag=f"bq{d}") for d in range(2)]
            bk_t = [pp.tile([128, 1], F32, name=f"bk{d}", tag=f"bk{d}") for d in range(2)]
            bo_t = [pp.tile([128, 1], F32, name=f"bo{e}", tag=f"bo{e}") for e in range(NET)]
            for d in range(2):
                nc.sync.dma_start(
                    bq_t[d][:], bq[d * 128:(d + 1) * 128].rearrange("(p one) -> p one", one=1))
                nc.sync.dma_start(
                    bk_t[d][:], bk[d * 128:(d + 1) * 128].rearrange("(p one) -> p one", one=1))
            for e in range(NET):
                nc.gpsimd.dma_start(
                    bo_t[e][:], bo[e * 128:(e + 1) * 128].rearrange("(p one) -> p one", one=1))
            # bv broadcast across partitions: [GD] -> [128, GD]
            bv_t = pp.tile([128, GD], F32, name="bv", tag="bv")
            nc.gpsimd.dma_start(bv_t[:], bv.ap().partition_broadcast(128))
            ones_f32 = pp.tile([128, 64], F32, name="ones_f32", tag="ones_f32")
            nc.vector.memset(ones_f32[:], 1.0)
            # mask row for the rcp broadcast matmul: [1, 128]
            mrow_t = pp.tile([1, 128], F32R, name="mrow", tag="mrow")
            nc.sync.dma_start(mrow_t[:], mrow.rearrange("(one p) -> one p", one=1))

            # persistent activations
            qt_sb = [pp.tile([128, TOK], F32R, name=f"qt{d}", tag=f"qt{d}") for d in range(2)]
            kt_sb = [pp.tile([128, TOK], F32R, name=f"kt{d}", tag=f"kt{d}") for d in range(2)]
            # V tok-major, packed [v_h | 1] per head: 65 cols per head
            vp_sb = [pp.tile([128, GH * 65], F32R, name=f"vp{t}", tag=f"vp{t}") for t in range(NKT)]
            for t in range(NKT):
                for h in range(GH):
                    nc.vector.tensor_copy(
                        vp_sb[t][:, h * 65 + 64:h * 65 + 65], ones_f32[:, 0:1])

            # A2A bounce buffers: ONE collective per rep carries all 4
            # heads: blocks [dest, 256, QB] bf16 (rows 64h..64h+63 = head h,
            # normalized O); dests 0:4 = batch-0 copy, 4:8 = batch-1 copy
            # (wrong-batch copy zeroed via the mask folded into the rcp
            # broadcast; the receiver's add merges). Ping-pong by rep parity
            # so the collective gets a full rep of pipeline slack.
            a2a_in = [dp.tile([NCORES, 4 * 64, QB], BF16, name=f"a2ain{s}", tag=f"a2ain{s}")
                      for s in range(2)]
            a2a_out = [dp.tile([NCORES, 4 * 64, QB], BF16, name=f"a2aout{s}", tag=f"a2aout{s}")
                       for s in range(2)]

            def load_x(tb):
                # ACT's DMA queue is free while x loads (exp hasn't started
                # for the tokens being loaded).
                xts = []
                for e in range(NET):
                    xt_t = xp.tile([128, QB], F32R, name="xt", tag=f"xt{e}")
                    eng = nc.sync if (e % 2 == 0) else nc.scalar
                    eng.dma_start(xt_t[:], xt[e * 128:(e + 1) * 128, tb * QB:(tb + 1) * QB])
                    xts.append(xt_t)
                return xts

            def phase1():
                # K+V for all tokens complete first (x tiles recycled per
                # tb); then Q per q-block with x re-loaded (extra 8.4MB DMA
                # on otherwise idle queues) so attention starts earlier.
                with tc.tile_pool(name="ps_qkv", bufs=1, space="PSUM") as ps_qkv:
                    for tb in range(NQB):
                        xts = load_x(tb)
                        for d in range(2):
                            ps = ps_qkv.tile([128, QB], F32, name=f"k{d}", tag=f"k{d}")
                            for e in range(NET):
                                mm = nc.tensor.matmul(
                                    ps[:], wk_t[e][:, d * 128:(d + 1) * 128], xts[e][:],
                                    start=(e == 0), stop=(e == NET - 1))
                                if tb == 0 and d == 0 and e == 0:
                                    MARKERS.setdefault("p1_K_first", mm.ins.name)
                                if tb == NQB - 1 and d == 1 and e == NET - 1:
                                    MARKERS.setdefault("p1_K_last", mm.ins.name)
                            nc.vector.tensor_scalar_add(
                                kt_sb[d][:, tb * QB:(tb + 1) * QB], ps[:], bk_t[d][:])
                        # V: tok-major [tok, dh]
                        for vt in range(4):
                            ps = ps_qkv.tile([128, GD], F32, name=f"v{vt % 2}", tag=f"v{vt % 2}")
                            for e in range(NET):
                                nc.tensor.matmul(
                                    ps[:], xts[e][:, vt * 128:(vt + 1) * 128], wv_t[e][:],
                                    start=(e == 0), stop=(e == NET - 1))
                            t = tb * 4 + vt
                            dst2 = vp_sb[t][:].rearrange("p (h c) -> p h c", h=GH)[:, :, 0:64]
                            nc.vector.tensor_tensor(
                                dst2, ps[:].rearrange("p (h c) -> p h c", h=GH),
                                bv_t[:].rearrange("p (h c) -> p h c", h=GH),
                                op=mybir.AluOpType.add)
                    for tb in range(NQB):
                        xts = load_x(tb)
                        for d in range(2):
                            ps = ps_qkv.tile([128, QB], F32, name=f"q{d}", tag=f"q{d}")
                            for e in range(NET):
                                mm = nc.tensor.matmul(
                                    ps[:], wq_t[e][:, d * 128:(d + 1) * 128], xts[e][:],
                                    start=(e == 0), stop=(e == NET - 1))
                                if tb == 0 and d == 1 and e == NET - 1:
                                    MARKERS.setdefault("p1_Q0_last", mm.ins.name)
                            nc.vector.tensor_scalar_add(
                                qt_sb[d][:, tb * QB:(tb + 1) * QB], ps[:], bq_t[d][:])

            prev_cc = {}
            prev_rd = {}

            def phase2(par):
                # per-head attention; each head's normalized bf16 output is
                # re-sharded head->token with its own small AllToAll so the
                # collectives start (and finish) as early as possible.
                with (
                    tc.tile_pool(name="ps_s", bufs=2, space="PSUM") as ps_s,
                    tc.tile_pool(name="ps_av", bufs=2, space="PSUM") as ps_av,
                    tc.tile_pool(name="ps_rb", bufs=1, space="PSUM") as ps_rb,
                ):
                    # k-tile groups sized to the scores psum tile (2 banks)
                    GRPS = [(k, 2) for k in range(0, NKT, 2)]
                    writers = []
                    for h in range(GH):
                        d, p0 = h // 2, (h % 2) * 64
                        for qb in range(NQB):
                            av_ps = ps_av.tile([65, QB], F32, name="av", tag="av")
                            for g0, gn in GRPS:
                                s_ps = ps_s.tile([128, 2 * QB], F32, name="s", tag="s")
                                for ki in range(gn):
                                    kt = g0 + ki
                                    nc.tensor.matmul(
                                        s_ps[:, ki * QB:(ki + 1) * QB],
                                        kt_sb[d][p0:p0 + 64, kt * 128:(kt + 1) * 128],
                                        qt_sb[d][p0:p0 + 64, qb * QB:(qb + 1) * QB],
                                        start=True, stop=True)
                                at_t = ap_.tile([128, 2 * QB], F32R, name="at", tag="at")
                                act = nc.scalar.activation(
                                    at_t[:, 0:gn * QB], s_ps[:, 0:gn * QB],
                                    FP.Exp, scale=float(SCALE))
                                MARKERS.setdefault(f"p2_exp_first_h{h}_q{qb}", act.ins.name)
                                for ki in range(gn):
                                    kt = g0 + ki
                                    nc.tensor.matmul(
                                        av_ps[:],
                                        vp_sb[kt][:, h * 65:h * 65 + 65],
                                        at_t[:, ki * QB:(ki + 1) * QB],
                                        start=(kt == 0), stop=(kt == NKT - 1))
                            # copy the accumulator to SBUF (frees the PSUM
                            # bank early, and TensorTensor may read at most
                            # one PSUM operand on hardware).
                            av_sb = op_.tile([65, QB], F32R, name="av_sb", tag="av_sb")
                            with nc.allow_low_precision(reason="tf32 av copy"):
                                nc.vector.tensor_copy(av_sb[:], av_ps[:])
                            # normalize + batch-mask on the sender:
                            # rcp = 1/rowsum [1, QB]; rb = mrow.T @ rcp
                            # [128, QB]: rows 0:64 = mlo*rcp, 64:128 =
                            # mhi*rcp (one of them all-zero).
                            rcp_t = op_.tile([1, QB], F32R, name="rcp", tag="rcp")
                            with nc.allow_low_precision(reason="tf32 rcp"):
                                nc.vector.reciprocal(rcp_t[:], av_sb[64:65, :])
                            rb_ps = ps_rb.tile([128, QB], F32, name="rb", tag="rb")
                            nc.tensor.matmul(
                                rb_ps[:], mrow_t[:], rcp_t[:], start=True, stop=True)
                            ot_lo = op_.tile([64, QB], BF16, name="ot_lo", tag="ot_lo")
                            ot_hi = op_.tile([64, QB], BF16, name="ot_hi", tag="ot_hi")
                            with nc.allow_low_precision(reason="bf16 a2a payload"):
                                nc.vector.tensor_tensor(
                                    ot_lo[:], av_sb[0:64, :], rb_ps[0:64, :],
                                    op=mybir.AluOpType.mult)
                                nc.vector.tensor_tensor(
                                    ot_hi[:], av_sb[0:64, :], rb_ps[64:128, :],
                                    op=mybir.AluOpType.mult)
                            w1 = nc.sync.dma_start(
                                a2a_in[par][qb, h * 64:(h + 1) * 64, :], ot_lo[:])
                            w2 = nc.sync.dma_start(
                                a2a_in[par][qb + 4, h * 64:(h + 1) * 64, :], ot_hi[:])
                            if par in prev_cc:
                                add_dep_helper(w1.ins, prev_cc[par].ins,
                                               reason="a2a_in reuse waits same-parity collective")
                                add_dep_helper(w2.ins, prev_cc[par].ins,
                                               reason="a2a_in reuse waits same-parity collective")
                            writers.append(w1)
                            writers.append(w2)
                    cc = nc.gpsimd.collective_compute(
                        "AllToAll", mybir.AluOpType.bypass,
                        replica_groups=[list(range(NCORES))],
                        ins=[a2a_in[par].opt()], outs=[a2a_out[par].opt()])
                    MARKERS.setdefault(f"p2_cc{par}", cc.ins.name)
                    for w in writers:
                        add_dep_helper(cc.ins, w.ins, reason="collective waits on a2a input writes")
                    for rdp in prev_rd.get(par, ()):
                        add_dep_helper(cc.ins, rdp.ins,
                                       reason="a2a_out overwrite waits same-parity prior reads")
                    prev_cc[par] = cc
                return cc

            def phase3(cc, par):
                with tc.tile_pool(name="ps_y", bufs=2, space="PSUM") as ps_y:
                    # otf_t[k] = O.T dims 128k..128k+127 for own tokens =
                    # local heads (2*(k%2), 2*(k%2)+1) from source group
                    # k//2; merge the two batch copies (wrong one is zero).
                    otf_t = [fp_.tile([128, QB], F32R, name=f"otf{k}", tag=f"otf{k}") for k in range(NET)]
                    prev_rd[par] = []
                    for k in range(NET):
                        g_src, h0 = k // 2, 2 * (k % 2)
                        ou_lo = fp_.tile([128, QB], BF16, name="ou_lo", tag="ou_lo", bufs=2)
                        ou_hi = fp_.tile([128, QB], BF16, name="ou_hi", tag="ou_hi", bufs=2)
                        rd1 = nc.gpsimd.dma_start(
                            ou_lo[:], a2a_out[par][g_src, h0 * 64:(h0 + 2) * 64, :])
                        rd2 = nc.gpsimd.dma_start(
                            ou_hi[:], a2a_out[par][4 + g_src, h0 * 64:(h0 + 2) * 64, :])
                        add_dep_helper(rd1.ins, cc.ins, reason="otf read waits on collective")
                        add_dep_helper(rd2.ins, cc.ins, reason="otf read waits on collective")
                        prev_rd[par].extend([rd1, rd2])
                        with nc.allow_low_precision(reason="bf16 -> tf32 merge"):
                            nc.vector.tensor_tensor(
                                otf_t[k][:], ou_lo[:], ou_hi[:], op=mybir.AluOpType.add)
                    for e in range(NET):
                        ps = ps_y.tile([128, QB], F32, name="y", tag="y")
                        for i, k in enumerate(range(NET)):
                            mm = nc.tensor.matmul(
                                ps[:], wo_t[k][:, e * 128:(e + 1) * 128], otf_t[k][:],
                                start=(i == 0), stop=(i == NET - 1))
                            if e == 0 and i == 0:
                                MARKERS.setdefault("p3_fc_first", mm.ins.name)
                        y_t = yp.tile([128, QB], F32, name="yt", tag="yt")
                        nc.vector.tensor_scalar_add(y_t[:], ps[:], bo_t[e][:])
                        w = nc.gpsimd.dma_start(yt[e * 128:(e + 1) * 128, :], y_t[:])
                        if e == NET - 1:
                            MARKERS.setdefault("p3_yt_last", w.ins.name)

            # Software pipeline across reps: rep r+1's QKV projections are
            # emitted BEFORE rep r's output projection, and rep r's output
            # projection is deferred past rep r+1's attention, giving the
            # single per-rep collective a full rep of pipeline slack.
            phase1()
            pend = None
            for r in range(reps):
                cc = phase2(r % 2)
                if r + 1 < reps:
                    phase1()
                if pend is not None:
                    phase3(*pend)
                pend = (cc, r % 2)
            phase3(*pend)

    nc.compile()
    return nc


_CACHE = {}


def _get_nc(reps: int = 1):
    if reps not in _CACHE:
        _CACHE[reps] = build_nc(reps)
    return _CACHE[reps]


def make_in_maps(x, Wq, bq, Wk, bk, Wv, bv, Wo, bo):
    x = np.asarray(x, np.float32)
    xts = [_tf32_round(np.ascontiguousarray(x[b].T)) for b in range(B)]
    wqt = _tf32_round(np.ascontiguousarray(np.asarray(Wq, np.float32).T))
    wkt = _tf32_round(np.ascontiguousarray(np.asarray(Wk, np.float32).T))
    wvt = _tf32_round(np.ascontiguousarray(np.asarray(Wv, np.float32).T))
    # Wo pre-divided by PAYLOAD_SCALE: the a2a payload ships O*PAYLOAD_SCALE
    wot = _tf32_round(np.ascontiguousarray(np.asarray(Wo, np.float32).T / PAYLOAD_SCALE))
    bq = np.asarray(bq, np.float32); bk = np.asarray(bk, np.float32)
    bv = np.asarray(bv, np.float32); bo = np.asarray(bo, np.float32)
    in_maps = []
    for c in range(NCORES):
        b, g = c // 4, c % 4
        sl = slice(g * GD, (g + 1) * GD)
        mrow = np.zeros(128, np.float32)
        mrow[0:64] = 1.0 if b == 0 else 0.0
        mrow[64:128] = 1.0 if b == 1 else 0.0
        in_maps.append({
            "mrow": mrow,
            "xt": xts[b],
            "wqt": np.ascontiguousarray(wqt[:, sl]),
            "wkt": np.ascontiguousarray(wkt[:, sl]),
            "wvt": np.ascontiguousarray(wvt[:, sl]),
            "wot": wot,
            "bq": np.ascontiguousarray(bq[sl]),
            "bk": np.ascontiguousarray(bk[sl]),
            "bv": np.ascontiguousarray(bv[sl]),
            "bo": bo,
        })
    return in_maps


def kernel(x, Wq, bq, Wk, bk, Wv, bv, Wo, bo):
    nc = _get_nc(1)
    in_maps = make_in_maps(x, Wq, bq, Wk, bk, Wv, bv, Wo, bo)
    res = run_bass_kernel_spmd(nc, in_maps, list(range(NCORES)))
    out = np.empty((B, S, E), np.float32)
    for c in range(NCORES):
        b, g = c // 4, c % 4
        out[b, g * QB:(g + 1) * QB, :] = res.results[c]["yt"].T
    return out


# revision 24
# speedup vs baseline: 2.4789x; 1.0339x over previous
"""Multi-head self-attention (B=2, S=2048, E=1024, H=16) on 8 TRN2 NeuronCores.

Sharding: core c handles batch b=c//4 and head group g=c%4 (4 heads each).
 - QKV projections are head-sharded (each core computes Q/K/V only for its
   4 heads, over all 2048 tokens of its batch) -> no K/V exchange needed.
 - Attention (scores -> exp -> AV) is fully local per core.
 - Attention output is NORMALIZED on the sender (reciprocal of the fused
   rowsum, broadcast across partitions via a mask-valued rank-1 matmul that
   also zeroes wrong-batch copies), converted to bf16, and re-sharded
   from head-sharding to token-sharding with one AllToAll per HEAD-PAIR
   (2 collectives of [8,128,512] bf16 instead of 4 of [8,65,512] f32).
 - Output projection (Wo) then runs token-sharded, producing complete
   output rows; the host just transposes/concatenates.

Everything feature-major ("transposed") on device: x, Q, K are [dim, tok]
so the PE's partition-contraction works without any on-device transposes;
weights are pre-transposed on the host. V is tok-major for the AV matmul,
with a fused ones-column producing softmax row-sums for free.

Matmuls run in float32r (TF32, 1 cycle/row at moving free>=256, vs 4 for
fp32). Inputs are pre-rounded to TF32 on the host so DMA-loaded operands
are valid fp32r. Softmax skips the max-subtraction (logits are ~N(0,1),
bounded ~|6|, exp is safe in fp32) and folds the 1/sqrt(64) scale into the
ACT exp instruction; normalization is applied before the collective using
the fused row-sums. Wo loads and yt stores ride the (otherwise idle) Pool
DMA queue so phase-1 x loads start immediately.
"""

import numpy as np

import concourse.bass as bass
import concourse.mybir as mybir
from concourse import tile, bacc
from concourse.tile import add_dep_helper
from concourse.bass_utils import run_bass_kernel_spmd

B = 2
S = 2048
E = 1024
H = 16
DH = 64

NCORES = 8
GH = 4          # heads per core
GD = GH * DH    # 256 feature dims per core
TOK = S         # tokens per core (its whole batch element)
QB = 512        # q-block (moving free dim)
NQB = TOK // QB         # 4
NKT = TOK // 128        # 16 k-tiles
NET = E // 128          # 8 e-tiles
SCALE = 1.0 / np.sqrt(DH)

F32 = mybir.dt.float32
F32R = mybir.dt.float32r
BF16 = mybir.dt.bfloat16
FP = mybir.ActivationFunctionType
PAYLOAD_SCALE = 1.0    # unused now that the payload is bf16


def _tf32_round(a: np.ndarray) -> np.ndarray:
    b = np.ascontiguousarray(a, dtype=np.float32).view(np.uint32)
    r = (b + np.uint32(0x1000) + ((b >> np.uint32(13)) & np.uint32(1))) & np.uint32(0xFFFFE000)
    return r.view(np.float32)


MARKERS = {}


def build_nc(reps: int = 1):
    MARKERS.clear()
    nc = bacc.Bacc("TRN2", target_bir_lowering=False, debug=False, num_devices=NCORES)

    xt = nc.dram_tensor("xt", [E, TOK], BF16, kind="ExternalInput")       # x[b].T
    wqt = nc.dram_tensor("wqt", [E, GD], BF16, kind="ExternalInput")      # Wq.T cols for group
    wkt = nc.dram_tensor("wkt", [E, GD], BF16, kind="ExternalInput")
    wvt = nc.dram_tensor("wvt", [E, GD], BF16, kind="ExternalInput")
    wot = nc.dram_tensor("wot", [E, E], BF16, kind="ExternalInput")       # Wo.T full
    bq = nc.dram_tensor("bq", [GD], F32, kind="ExternalInput")
    bk = nc.dram_tensor("bk", [GD], F32, kind="ExternalInput")
    bv = nc.dram_tensor("bv", [GD], F32, kind="ExternalInput")
    bo = nc.dram_tensor("bo", [E], F32, kind="ExternalInput")
    # mask row [128]: cols 0:64 = 1.0 iff this core is batch-0, cols
    # 64:128 = 1.0 iff batch-1. lhsT of the rcp-broadcast matmul.
    mrow = nc.dram_tensor("mrow", [128], F32R, kind="ExternalInput")
    yt = nc.dram_tensor("yt", [E, QB], BF16, kind="ExternalOutput")       # out rows, transposed

    with tile.TileContext(nc) as tc:
        with (
            tc.tile_pool(name="weights", bufs=1) as wp,
            tc.tile_pool(name="persist", bufs=1) as pp,
            tc.tile_pool(name="xt", bufs=2) as xp,
            tc.tile_pool(name="at", bufs=3) as ap_,
            tc.tile_pool(name="ot", bufs=4) as op_,
            tc.tile_pool(name="otf", bufs=1) as fp_,
            tc.tile_pool(name="yt", bufs=2) as yp,
            tc.tile_pool(name="dram", bufs=1, space="DRAM") as dp,
        ):
            # ---- persistent weights/biases in SBUF ----
            # qkv weights on the sync queue (needed first); wo on the Pool
            # queue so it never delays the phase-1 x loads.
            wq_t = [wp.tile([128, GD], BF16, name=f"wq{e}", tag=f"wq{e}") for e in range(NET)]
            wk_t = [wp.tile([128, GD], BF16, name=f"wk{e}", tag=f"wk{e}") for e in range(NET)]
            wv_t = [wp.tile([128, GD], BF16, name=f"wv{e}", tag=f"wv{e}") for e in range(NET)]
            wo_t = [wp.tile([128, E], BF16, name=f"wo{k}", tag=f"wo{k}") for k in range(NET)]
            for e in range(NET):
                nc.sync.dma_start(wk_t[e][:], wkt[e * 128:(e + 1) * 128, :])
                nc.sync.dma_start(wq_t[e][:], wqt[e * 128:(e + 1) * 128, :])
                nc.sync.dma_start(wv_t[e][:], wvt[e * 128:(e + 1) * 128, :])
            for e in range(NET):
                nc.gpsimd.dma_start(wo_t[e][:], wot[e * 128:(e + 1) * 128, :])

            bq_t = [pp.tile([128, 1], F32, name=f"bq{d}", tag=f"bq{d}") for d in range(2)]
            bk_t = [pp.tile([128, 1], F32, name=f"bk{d}", tag=f"bk{d}") for d in range(2)]
            bo_t = [pp.tile([128, 1], F32, name=f"bo{e}", tag=f"bo{e}") for e in range(NET)]
            for d in range(2):
                nc.sync.dma_start(
                    bq_t[d][:], bq[d * 128:(d + 1) * 128].rearrange("(p one) -> p one", one=1))
                nc.sync.dma_start(
                    bk_t[d][:], bk[d * 128:(d + 1) * 128].rearrange("(p one) -> p one", one=1))
            for e in range(NET):
                nc.gpsimd.dma_start(
                    bo_t[e][:], bo[e * 128:(e + 1) * 128].rearrange("(p one) -> p one", one=1))
            # bv broadcast across partitions: [GD] -> [128, GD]
            bv_t = pp.tile([128, GD], F32, name="bv", tag="bv")
            nc.gpsimd.dma_start(bv_t[:], bv.ap().partition_broadcast(128))
            ones_f32 = pp.tile([128, 64], F32, name="ones_f32", tag="ones_f32")
            nc.vector.memset(ones_f32[:], 1.0)
            # mask row for the rcp broadcast matmul: [1, 128]
            mrow_t = pp.tile([1, 128], F32R, name="mrow", tag="mrow")
            nc.sync.dma_start(mrow_t[:], mrow.rearrange("(one p) -> one p", one=1))

            # persistent activations
            qt_sb = [pp.tile([128, TOK], F32R, name=f"qt{d}", tag=f"qt{d}") for d in range(2)]
            kt_sb = [pp.tile([128, TOK], F32R, name=f"kt{d}", tag=f"kt{d}") for d in range(2)]
            # V tok-major, packed [v_h | 1] per head: 65 cols per head
            vp_sb = [pp.tile([128, GH * 65], F32R, name=f"vp{t}", tag=f"vp{t}") for t in range(NKT)]
            for t in range(NKT):
                for h in range(GH):
                    nc.vector.tensor_copy(
                        vp_sb[t][:, h * 65 + 64:h * 65 + 65], ones_f32[:, 0:1])

            # A2A bounce buffers: ONE collective per rep carries all 4
            # heads: blocks [dest, 256, QB] bf16 (rows 64h..64h+63 = head h,
            # normalized O); dests 0:4 = batch-0 copy, 4:8 = batch-1 copy
            # (wrong-batch copy zeroed via the mask folded into the rcp
            # broadcast; the receiver's add merges). Ping-pong by rep parity
            # so the collective gets a full rep of pipeline slack.
            a2a_in = [dp.tile([NCORES, 4 * 64, QB], BF16, name=f"a2ain{s}", tag=f"a2ain{s}")
                      for s in range(2)]
            a2a_out = [dp.tile([NCORES, 4 * 64, QB], BF16, name=f"a2aout{s}", tag=f"a2aout{s}")
                       for s in range(2)]

            def load_x(tb):
                # ACT's DMA queue is free while x loads (exp hasn't started
                # for the tokens being loaded).
                xts = []
                for e in range(NET):
                    xt_t = xp.tile([128, QB], BF16, name="xt", tag=f"xt{e}")
                    eng = nc.sync if (e % 2 == 0) else nc.scalar
                    eng.dma_start(xt_t[:], xt[e * 128:(e + 1) * 128, tb * QB:(tb + 1) * QB])
                    xts.append(xt_t)
                return xts

            def phase1():
                # one pass over x: K and V then Q per token-block, x tiles
                # loaded once (bf16) and recycled.
                with tc.tile_pool(name="ps_qkv", bufs=1, space="PSUM") as ps_qkv:
                    for tb in range(NQB):
                        xts = load_x(tb)
                        for d in range(2):
                            ps = ps_qkv.tile([128, QB], F32, name=f"k{d}", tag=f"k{d}")
                            for e in range(NET):
                                mm = nc.tensor.matmul(
                                    ps[:], wk_t[e][:, d * 128:(d + 1) * 128], xts[e][:],
                                    start=(e == 0), stop=(e == NET - 1))
                                if tb == 0 and d == 0 and e == 0:
                                    MARKERS.setdefault("p1_K_first", mm.ins.name)
                                if tb == NQB - 1 and d == 1 and e == NET - 1:
                                    MARKERS.setdefault("p1_K_last", mm.ins.name)
                            nc.vector.tensor_scalar_add(
                                kt_sb[d][:, tb * QB:(tb + 1) * QB], ps[:], bk_t[d][:])
                        # V: tok-major [tok, dh]
                        for vt in range(4):
                            ps = ps_qkv.tile([128, GD], F32, name=f"v{vt % 2}", tag=f"v{vt % 2}")
                            for e in range(NET):
                                nc.tensor.matmul(
                                    ps[:], xts[e][:, vt * 128:(vt + 1) * 128], wv_t[e][:],
                                    start=(e == 0), stop=(e == NET - 1))
                            t = tb * 4 + vt
                            dst2 = vp_sb[t][:].rearrange("p (h c) -> p h c", h=GH)[:, :, 0:64]
                            nc.vector.tensor_tensor(
                                dst2, ps[:].rearrange("p (h c) -> p h c", h=GH),
                                bv_t[:].rearrange("p (h c) -> p h c", h=GH),
                                op=mybir.AluOpType.add)
                        for d in range(2):
                            ps = ps_qkv.tile([128, QB], F32, name=f"q{d}", tag=f"q{d}")
                            for e in range(NET):
                                mm = nc.tensor.matmul(
                                    ps[:], wq_t[e][:, d * 128:(d + 1) * 128], xts[e][:],
                                    start=(e == 0), stop=(e == NET - 1))
                                if tb == 0 and d == 1 and e == NET - 1:
                                    MARKERS.setdefault("p1_Q0_last", mm.ins.name)
                            nc.vector.tensor_scalar_add(
                                qt_sb[d][:, tb * QB:(tb + 1) * QB], ps[:], bq_t[d][:])

            prev_cc = {}
            prev_rd = {}

            def phase2(par):
                # per-head attention; each head's normalized bf16 output is
                # re-sharded head->token with its own small AllToAll so the
                # collectives start (and finish) as early as possible.
                with (
                    tc.tile_pool(name="ps_s", bufs=2, space="PSUM") as ps_s,
                    tc.tile_pool(name="ps_av", bufs=2, space="PSUM") as ps_av,
                    tc.tile_pool(name="ps_rb", bufs=1, space="PSUM") as ps_rb,
                ):
                    # k-tile groups sized to the scores psum tile (2 banks)
                    GRPS = [(k, 2) for k in range(0, NKT, 2)]
                    writers = []
                    for h in range(GH):
                        d, p0 = h // 2, (h % 2) * 64
                        for qb in range(NQB):
                            av_ps = ps_av.tile([65, QB], F32, name="av", tag="av")
                            for g0, gn in GRPS:
                                s_ps = ps_s.tile([128, 2 * QB], F32, name="s", tag="s")
                                for ki in range(gn):
                                    kt = g0 + ki
                                    nc.tensor.matmul(
                                        s_ps[:, ki * QB:(ki + 1) * QB],
                                        kt_sb[d][p0:p0 + 64, kt * 128:(kt + 1) * 128],
                                        qt_sb[d][p0:p0 + 64, qb * QB:(qb + 1) * QB],
                                        start=True, stop=True)
                                at_t = ap_.tile([128, 2 * QB], F32R, name="at", tag="at")
                                act = nc.scalar.activation(
                                    at_t[:, 0:gn * QB], s_ps[:, 0:gn * QB],
                                    FP.Exp, scale=float(SCALE))
                                MARKERS.setdefault(f"p2_exp_first_h{h}_q{qb}", act.ins.name)
                                for ki in range(gn):
                                    kt = g0 + ki
                                    nc.tensor.matmul(
                                        av_ps[:],
                                        vp_sb[kt][:, h * 65:h * 65 + 65],
                                        at_t[:, ki * QB:(ki + 1) * QB],
                                        start=(kt == 0), stop=(kt == NKT - 1))
                            # copy the accumulator to SBUF (frees the PSUM
                            # bank early, and TensorTensor may read at most
                            # one PSUM operand on hardware).
                            av_sb = op_.tile([65, QB], F32R, name="av_sb", tag="av_sb")
                            with nc.allow_low_precision(reason="tf32 av copy"):
                                nc.vector.tensor_copy(av_sb[:], av_ps[:])
                            # normalize + batch-mask on the sender:
                            # rcp = 1/rowsum [1, QB]; rb = mrow.T @ rcp
                            # [128, QB]: rows 0:64 = mlo*rcp, 64:128 =
                            # mhi*rcp (one of them all-zero).
                            rcp_t = op_.tile([1, QB], F32R, name="rcp", tag="rcp")
                            with nc.allow_low_precision(reason="tf32 rcp"):
                                nc.vector.reciprocal(rcp_t[:], av_sb[64:65, :])
                            rb_ps = ps_rb.tile([128, QB], F32, name="rb", tag="rb")
                            nc.tensor.matmul(
                                rb_ps[:], mrow_t[:], rcp_t[:], start=True, stop=True)
                            ot_lo = op_.tile([64, QB], BF16, name="ot_lo", tag="ot_lo")
                            ot_hi = op_.tile([64, QB], BF16, name="ot_hi", tag="ot_hi")
                            with nc.allow_low_precision(reason="bf16 a2a payload"):
                                nc.vector.tensor_tensor(
                                    ot_lo[:], av_sb[0:64, :], rb_ps[0:64, :],
                                    op=mybir.AluOpType.mult)
                                nc.vector.tensor_tensor(
                                    ot_hi[:], av_sb[0:64, :], rb_ps[64:128, :],
                                    op=mybir.AluOpType.mult)
                            w1 = nc.sync.dma_start(
                                a2a_in[par][qb, h * 64:(h + 1) * 64, :], ot_lo[:])
                            w2 = nc.sync.dma_start(
                                a2a_in[par][qb + 4, h * 64:(h + 1) * 64, :], ot_hi[:])
                            if par in prev_cc:
                                add_dep_helper(w1.ins, prev_cc[par].ins,
                                               reason="a2a_in reuse waits same-parity collective")
                                add_dep_helper(w2.ins, prev_cc[par].ins,
                                               reason="a2a_in reuse waits same-parity collective")
                            writers.append(w1)
                            writers.append(w2)
                    cc = nc.gpsimd.collective_compute(
                        "AllToAll", mybir.AluOpType.bypass,
                        replica_groups=[list(range(NCORES))],
                        ins=[a2a_in[par].opt()], outs=[a2a_out[par].opt()])
                    MARKERS.setdefault(f"p2_cc{par}", cc.ins.name)
                    for w in writers:
                        add_dep_helper(cc.ins, w.ins, reason="collective waits on a2a input writes")
                    for rdp in prev_rd.get(par, ()):
                        add_dep_helper(cc.ins, rdp.ins,
                                       reason="a2a_out overwrite waits same-parity prior reads")
                    prev_cc[par] = cc
                return cc

            def phase3(cc, par):
                with tc.tile_pool(name="ps_y", bufs=2, space="PSUM") as ps_y:
                    # otf_t[k] = O.T dims 128k..128k+127 for own tokens =
                    # local heads (2*(k%2), 2*(k%2)+1) from source group
                    # k//2; merge the two batch copies (wrong one is zero).
                    otf_t = [fp_.tile([128, QB], BF16, name=f"otf{k}", tag=f"otf{k}") for k in range(NET)]
                    prev_rd[par] = []
                    for k in range(NET):
                        g_src, h0 = k // 2, 2 * (k % 2)
                        ou_lo = fp_.tile([128, QB], BF16, name="ou_lo", tag="ou_lo", bufs=2)
                        ou_hi = fp_.tile([128, QB], BF16, name="ou_hi", tag="ou_hi", bufs=2)
                        rd1 = nc.gpsimd.dma_start(
                            ou_lo[:], a2a_out[par][g_src, h0 * 64:(h0 + 2) * 64, :])
                        rd2 = nc.gpsimd.dma_start(
                            ou_hi[:], a2a_out[par][4 + g_src, h0 * 64:(h0 + 2) * 64, :])
                        add_dep_helper(rd1.ins, cc.ins, reason="otf read waits on collective")
                        add_dep_helper(rd2.ins, cc.ins, reason="otf read waits on collective")
                        prev_rd[par].extend([rd1, rd2])
                        with nc.allow_low_precision(reason="bf16 -> tf32 merge"):
                            nc.vector.tensor_tensor(
                                otf_t[k][:], ou_lo[:], ou_hi[:], op=mybir.AluOpType.add)
                    for e in range(NET):
                        ps = ps_y.tile([128, QB], F32, name="y", tag="y")
                        for i, k in enumerate(range(NET)):
                            mm = nc.tensor.matmul(
                                ps[:], wo_t[k][:, e * 128:(e + 1) * 128], otf_t[k][:],
                                start=(i == 0), stop=(i == NET - 1))
                            if e == 0 and i == 0:
                                MARKERS.setdefault("p3_fc_first", mm.ins.name)
                        y_t = yp.tile([128, QB], BF16, name="yt", tag="yt")
                        nc.vector.tensor_scalar_add(y_t[:], ps[:], bo_t[e][:])
                        w = nc.gpsimd.dma_start(yt[e * 128:(e + 1) * 128, :], y_t[:])
                        if e == NET - 1:
                            MARKERS.setdefault("p3_yt_last", w.ins.name)

            # Software pipeline across reps: rep r+1's QKV projections are
            # emitted BEFORE rep r's output projection, and rep r's output
            # projection is deferred past rep r+1's attention, giving the
            # single per-rep collective a full rep of pipeline slack.
            phase1()
            pend = None
            for r in range(reps):
                cc = phase2(r % 2)
                if r + 1 < reps:
                    phase1()
                if pend is not None:
                    phase3(*pend)
                pend = (cc, r % 2)
            phase3(*pend)

    nc.compile()
    return nc


_CACHE = {}


def _get_nc(reps: int = 1):
    if reps not in _CACHE:
        _CACHE[reps] = build_nc(reps)
    return _CACHE[reps]


def make_in_maps(x, Wq, bq, Wk, bk, Wv, bv, Wo, bo):
    import ml_dtypes
    bf = ml_dtypes.bfloat16
    x = np.asarray(x, np.float32)
    xts = [np.ascontiguousarray(x[b].T).astype(bf) for b in range(B)]
    wqt = np.ascontiguousarray(np.asarray(Wq, np.float32).T).astype(bf)
    wkt = np.ascontiguousarray(np.asarray(Wk, np.float32).T).astype(bf)
    wvt = np.ascontiguousarray(np.asarray(Wv, np.float32).T).astype(bf)
    wot = np.ascontiguousarray(np.asarray(Wo, np.float32).T).astype(bf)
    bq = np.asarray(bq, np.float32); bk = np.asarray(bk, np.float32)
    bv = np.asarray(bv, np.float32); bo = np.asarray(bo, np.float32)
    in_maps = []
    for c in range(NCORES):
        b, g = c // 4, c % 4
        sl = slice(g * GD, (g + 1) * GD)
        mrow = np.zeros(128, np.float32)
        mrow[0:64] = 1.0 if b == 0 else 0.0
        mrow[64:128] = 1.0 if b == 1 else 0.0
        in_maps.append({
            "mrow": mrow,
            "xt": xts[b],
            "wqt": np.ascontiguousarray(wqt[:, sl]),
            "wkt": np.ascontiguousarray(wkt[:, sl]),
            "wvt": np.ascontiguousarray(wvt[:, sl]),
            "wot": wot,
            "bq": np.ascontiguousarray(bq[sl]),
            "bk": np.ascontiguousarray(bk[sl]),
            "bv": np.ascontiguousarray(bv[sl]),
            "bo": bo,
        })
    return in_maps


def kernel(x, Wq, bq, Wk, bk, Wv, bv, Wo, bo):
    nc = _get_nc(1)
    in_maps = make_in_maps(x, Wq, bq, Wk, bk, Wv, bv, Wo, bo)
    res = run_bass_kernel_spmd(nc, in_maps, list(range(NCORES)))
    out = np.empty((B, S, E), np.float32)
    for c in range(NCORES):
        b, g = c // 4, c % 4
        out[b, g * QB:(g + 1) * QB, :] = res.results[c]["yt"].T.astype(np.float32)
    return out
